# revision 38
# baseline (speedup 1.0000x reference)
"""Trainium2 Bass kernel for nn_CLloss (contrastive loss, anchor row 0).

Math (faithful to the torch/jax reference):
    e_j = x_j / max(||x_j||, 1e-12)          (row-normalize embed)
    d_j = ||(e_0 + 1e-6) - e_j||_2           (pairwise distance to anchor, j>=1)
    log_sim_j = -d_j / 0.1
    c_j = <labels_j, labels_0>
    Ci = 1e-12 + sum c_j ; Ei = 1e-12 + sum exp(log_sim_j)
    Li = sum -(c_j/Ci) * (log_sim_j - log Ei) ; loss = Li / n

With a = e_0 + 1e-6:  d_j^2 = ||a||^2 + 1 - 2*(a . x_j)/||x_j||, so the only
O(n*d) work is two per-row contractions over the feature dim: a.x_j and
sum_k x_jk^2.  Rows are sharded across 8 cores; each core gets its shard
TRANSPOSED (feature k on SBUF partitions, done on host) so the tensor engine
contracts over partitions:
  - a.x     via matmul(lhsT=[a | 0],  rhs=x)
  - sum x^2 via matmul(lhsT=[0 | 1],  rhs=square(x))
Both accumulate into the SAME psum tile (row 0 = a.x, row 1 = sum x^2)
across the feature chunks.  Squares are split between the scalar and vector
engines.  Inputs are cast to fp8 e4m3 on the host and matmuls use the
DoubleRow perf mode (256-deep contraction, 2 rows/cycle), which halves both
HBM traffic and tensor-engine time vs bf16.

Precision: the loss is a mean over 16k rows, so independent per-row rounding
noise averages down by ~sqrt(16384), and the fp8 quantization of the shared
anchor shifts all distances nearly uniformly — a shift that cancels exactly
between the sum(c*d)/T term and log(Ei).  Measured end-to-end error vs the
f32 reference is ~1e-5.  Device returns per-row (a.x, sum x^2); host does
the O(n) epilogue in f64.
"""

import ml_dtypes
import numpy as np

import concourse.bacc as bacc
import concourse.bass as bass
import concourse.tile as tile
from concourse import mybir
from concourse.bass_utils import run_bass_kernel_spmd

N_ROWS = 16384
DIM = 2048
N_CORES = 8
ROWS_PER_CORE = N_ROWS // N_CORES  # 2048
KC = DIM // 128  # 16 feature chunks of 128 partitions
KP = KC // 2  # 8 chunk-pairs (DoubleRow contracts 256 rows per matmul)
JC = ROWS_PER_CORE // 512  # 4 row chunks of 512 (psum bank = 512 f32)

PD_EPS = 1e-6
NORM_EPS = 1e-12
T = 0.1

FP8 = ml_dtypes.float8_e4m3

_NC_CACHE = {}


def _build_bass():
    # Bacc (not raw Bass): its compile() legalizes sync waits — walrus accepts
    # at most ONE wait per instruction, and Tile freely emits several.
    nc = bacc.Bacc()
    f32 = mybir.dt.float32
    fp8 = mybir.dt.float8e4
    xt = nc.dram_tensor("xt", [DIM, ROWS_PER_CORE], fp8, kind="ExternalInput")
    # Per chunk-pair p and pass wtype (0 = x, 1 = x^2), a [128, 2, 16] weight
    # block (DoubleRow ldweights requires the pair dim stride to be a
    # multiple of 16 elements).  Useful columns: m=0 carries a_chunk for the
    # x-pass, m=1 carries ones for the x^2-pass; the rest are zero.  Both
    # passes accumulate into the SAME psum tile: row 0 collects a.x only,
    # row 1 collects sum x^2 only.
    aw = nc.dram_tensor("aw", [128, 64 * KP], fp8, kind="ExternalInput")
    # columns [0:ROWS]: accumulator set A (row 0 = a.x, row 1 = sum x^2)
    # over even chunk-pairs; columns [ROWS:2*ROWS]: set B over odd pairs.
    # Host adds the two sets.  Two independent psum chains per row-block
    # keep one late square from stalling the other half of the matmul
    # stream.
    out = nc.dram_tensor(
        "out", [2, 2 * ROWS_PER_CORE], f32, kind="ExternalOutput"
    )

    # view as chunk-pairs: pair p, partition q, free [b, j] with b in {0,1}
    xt_pairs = xt.rearrange("(p b q) j -> p q b j", b=2, q=128)

    with tile.TileContext(nc) as tc:
        with (
            tc.tile_pool(name="xp", bufs=8) as xp,
            tc.tile_pool(name="x0p", bufs=4) as x0p,
            tc.tile_pool(name="singles", bufs=1) as singles,
            tc.tile_pool(name="psum", bufs=1, space="PSUM") as psum,
        ):
            aw_sb = singles.tile([128, 64 * KP], fp8)
            nc.sync.dma_start(out=aw_sb[:], in_=aw[:])
            aw_view = aw_sb.rearrange(
                "q (p w b m) -> q p w b m", p=KP, w=2, b=2
            )

            ps = {
                (s, j): psum.tile(
                    [16, 512], f32, tag=f"ps{s}{j}", name=f"ps{s}{j}"
                )
                for s in range(2)
                for j in range(JC)
            }

            def mm(out_ap, w, rhs, start, stop):
                nc.tensor.matmul(
                    out_ap,
                    w,
                    rhs,
                    start=start,
                    stop=stop,
                    perf_mode=mybir.MatmulPerfMode.DoubleRow,
                )

            def w_slices(p):
                return aw_view[:, p, 0], aw_view[:, p, 1]  # [128, 2, 16]

            # Segments: pair 0 is split into four 128 KB sub-tiles so the
            # first matmuls start as soon as the first sub-transfer lands
            # (warming the PE clock on real work); the last pair is split in
            # two so the tail square->matmul chain is short; the rest are
            # full 512 KB pair tiles.  (pair, j_lo, j_width)
            segments = [(0, j * 512, 512) for j in range(JC)]
            segments += [(p, 0, ROWS_PER_CORE) for p in range(1, KP - 1)]
            segments += [(KP - 1, h * 1024, 1024) for h in range(2)]

            for p, j_lo, j_w in segments:
                is_sub = j_w != ROWS_PER_CORE
                pool = x0p if is_sub else xp
                x_tile = pool.tile(
                    [128, 2, j_w], fp8, tag="x0" if is_sub else "x",
                    name=f"x_{p}_{j_lo}",
                )
                nc.sync.dma_start(
                    out=x_tile[:],
                    in_=xt_pairs[p][:, :, j_lo : j_lo + j_w],
                )
                sq_tile = pool.tile(
                    [128, 2, j_w], fp8, tag="sq0" if is_sub else "sq",
                    name=f"sq_{p}_{j_lo}",
                )
                # squares: split across scalar / vector / gpsimd by their
                # measured fp8 elementwise rates.
                if p == 6:
                    b0_eng = "v"
                elif p in (2, 4):
                    b0_eng = "s"
                else:
                    b0_eng = "s"
                b1_eng = "g" if p in (2, 4) else "v"
                for b, eng in ((0, b0_eng), (1, b1_eng)):
                    dst, src = sq_tile[:, b, :], x_tile[:, b, :]
                    if eng == "s":
                        nc.scalar.activation(
                            out=dst, in_=src,
                            func=mybir.ActivationFunctionType.Square,
                        )
                    elif eng == "v":
                        nc.vector.tensor_mul(dst, src, src)
                    else:
                        nc.gpsimd.tensor_mul(dst, src, src)
                s = p % 2
                first = p == s  # pair 0 starts set A, pair 1 starts set B
                last = p >= KP - 2  # pairs 6 (A) and 7 (B) close their sets
                w_x, w_q = w_slices(p)
                njc = j_w // 512
                for j in range(njc):
                    mm(
                        ps[s, j_lo // 512 + j][:],
                        w_x,
                        x_tile[:, :, j * 512 : (j + 1) * 512],
                        start=first,
                        stop=False,
                    )
                for j in range(njc):
                    mm(
                        ps[s, j_lo // 512 + j][:],
                        w_q,
                        sq_tile[:, :, j * 512 : (j + 1) * 512],
                        start=False,
                        stop=last,
                    )

            out_sb = singles.tile([2, 2 * ROWS_PER_CORE], f32)
            for s in range(2):
                for j in range(JC):
                    lo = s * ROWS_PER_CORE + j * 512
                    dst = out_sb[0:2, lo : lo + 512]
                    if (s + j) % 2 == 0:
                        nc.vector.tensor_copy(dst, ps[s, j][0:2, :])
                    else:
                        nc.scalar.copy(dst, ps[s, j][0:2, :])
            nc.sync.dma_start(out=out[:], in_=out_sb[:])

    nc.compile()
    return nc


def _get_nc():
    if "nc" not in _NC_CACHE:
        _NC_CACHE["nc"] = _build_bass()
    return _NC_CACHE["nc"]


def _make_in_maps(embed):
    x0 = embed[0].astype(np.float64)
    nrm0 = max(np.sqrt(np.dot(x0, x0)), NORM_EPS)
    a64 = x0 / nrm0 + PD_EPS
    a8 = a64.astype(FP8)

    # [128, p, wtype, b, m=16]: wtype 0 m=0 -> a_chunk, wtype 1 m=1 -> 1.0
    aw = np.zeros((128, KP, 2, 2, 16), FP8)
    for p in range(KP):
        for b in range(2):
            c = 2 * p + b
            aw[:, p, 0, b, 0] = a8[c * 128 : (c + 1) * 128]
            aw[:, p, 1, b, 1] = 1.0
    aw = aw.reshape(128, 64 * KP)

    in_maps = []
    for core in range(N_CORES):
        shard = embed[core * ROWS_PER_CORE : (core + 1) * ROWS_PER_CORE]
        xt = shard.T.astype(FP8)  # [DIM, ROWS_PER_CORE], C-contiguous
        in_maps.append({"xt": xt, "aw": aw})
    return in_maps, a64


def _epilogue(results, a64, labels):
    R = ROWS_PER_CORE
    adot = np.concatenate(
        [r["out"][0, :R].astype(np.float64) + r["out"][0, R:] for r in results]
    )
    ss = np.concatenate(
        [r["out"][1, :R].astype(np.float64) + r["out"][1, R:] for r in results]
    )

    nrm = np.maximum(np.sqrt(ss), NORM_EPS)
    t = adot / nrm  # a . e_j
    a2 = np.dot(a64, a64)
    d2 = np.maximum(a2 + 1.0 - 2.0 * t, 0.0)
    d = np.sqrt(d2)[1:]  # anchor row excluded, j = 1..n-1

    lab = labels.astype(np.float64)
    c = lab[1:] @ lab[0]
    ci = 1e-12 + c.sum()
    log_sim = -d / T
    ei = 1e-12 + np.exp(log_sim).sum()
    li = (-(c / ci) * (log_sim - np.log(ei))).sum()
    return np.asarray(li / N_ROWS, dtype=np.float32)


def _run(embed, labels, trace=False):
    embed = np.ascontiguousarray(np.asarray(embed, dtype=np.float32))
    labels = np.asarray(labels)
    assert embed.shape == (N_ROWS, DIM), embed.shape

    nc = _get_nc()
    in_maps, a64 = _make_in_maps(embed)
    kwargs = {"trace_cores": list(range(N_CORES))} if trace else {}
    res = run_bass_kernel_spmd(
        nc, in_maps, core_ids=list(range(N_CORES)), trace=trace, **kwargs
    )
    return _epilogue(res.results, a64, labels), res


def kernel(embed, labels):
    out, _ = _run(embed, labels, trace=False)
    return out


# revision 39
# speedup vs baseline: 1.1316x; 1.1316x over previous
"""Trainium2 Bass kernel for nn_CLloss (contrastive loss, anchor row 0).

Math (faithful to the torch/jax reference):
    e_j = x_j / max(||x_j||, 1e-12)          (row-normalize embed)
    d_j = ||(e_0 + 1e-6) - e_j||_2           (pairwise distance to anchor, j>=1)
    log_sim_j = -d_j / 0.1
    c_j = <labels_j, labels_0>
    Ci = 1e-12 + sum c_j ; Ei = 1e-12 + sum exp(log_sim_j)
    Li = sum -(c_j/Ci) * (log_sim_j - log Ei) ; loss = Li / n

With a = e_0 + 1e-6:  d_j^2 = ||a||^2 + 1 - 2*(a . x_j)/||x_j||, so the only
O(n*d) work is two per-row contractions over the feature dim: a.x_j and
sum_k x_jk^2.  Rows are sharded across 8 cores; each core gets its shard
TRANSPOSED (feature k on SBUF partitions, done on host) so the tensor engine
contracts over partitions:
  - a.x     via matmul(lhsT=[a | 0],  rhs=x)
  - sum x^2 via matmul(lhsT=[0 | 1],  rhs=square(x))
Both accumulate into the SAME psum tile (row 0 = a.x, row 1 = sum x^2)
across the feature chunks.  Squares are split between the scalar and vector
engines.  Inputs are cast to fp8 e4m3 on the host and matmuls use the
DoubleRow perf mode (256-deep contraction, 2 rows/cycle), which halves both
HBM traffic and tensor-engine time vs bf16.

Precision: the loss is a mean over 16k rows, so independent per-row rounding
noise averages down by ~sqrt(16384), and the fp8 quantization of the shared
anchor shifts all distances nearly uniformly — a shift that cancels exactly
between the sum(c*d)/T term and log(Ei).  Measured end-to-end error vs the
f32 reference is ~1e-5.  Device returns per-row (a.x, sum x^2); host does
the O(n) epilogue in f64.
"""

import ml_dtypes
import numpy as np

import concourse.bacc as bacc
import concourse.bass as bass
import concourse.tile as tile
from concourse import mybir
from concourse.bass_utils import run_bass_kernel_spmd

N_ROWS = 16384
DIM = 2048
N_CORES = 8
ROWS_PER_CORE = N_ROWS // N_CORES  # 2048
KC = DIM // 128  # 16 feature chunks of 128 partitions
KP = KC // 2  # 8 chunk-pairs (DoubleRow contracts 256 rows per matmul)
JC = ROWS_PER_CORE // 512  # 4 row chunks of 512 (psum bank = 512 f32)

PD_EPS = 1e-6
NORM_EPS = 1e-12
T = 0.1

FP8 = ml_dtypes.float8_e4m3

_NC_CACHE = {}


def _build_bass():
    # Bacc (not raw Bass): its compile() legalizes sync waits — walrus accepts
    # at most ONE wait per instruction, and Tile freely emits several.
    nc = bacc.Bacc()
    f32 = mybir.dt.float32
    fp8 = mybir.dt.float8e4
    xt = nc.dram_tensor("xt", [DIM, ROWS_PER_CORE], fp8, kind="ExternalInput")
    # Per chunk-pair p and pass wtype (0 = x, 1 = x^2), a [128, 2, 16] weight
    # block (DoubleRow ldweights requires the pair dim stride to be a
    # multiple of 16 elements).  Useful columns: m=0 carries a_chunk for the
    # x-pass, m=1 carries ones for the x^2-pass; the rest are zero.  Both
    # passes accumulate into the SAME psum tile: row 0 collects a.x only,
    # row 1 collects sum x^2 only.
    aw = nc.dram_tensor("aw", [128, 64 * KP], fp8, kind="ExternalInput")
    # columns [0:ROWS]: accumulator set A (row 0 = a.x, row 1 = sum x^2)
    # over even chunk-pairs; columns [ROWS:2*ROWS]: set B over odd pairs.
    # Host adds the two sets.  Two independent psum chains per row-block
    # keep one late square from stalling the other half of the matmul
    # stream.
    out = nc.dram_tensor(
        "out", [2, 2 * ROWS_PER_CORE], f32, kind="ExternalOutput"
    )

    # view as chunk-pairs: pair p, partition q, free [b, j] with b in {0,1}
    xt_pairs = xt.rearrange("(p b q) j -> p q b j", b=2, q=128)

    with tile.TileContext(nc) as tc:
        with (
            tc.tile_pool(name="xp", bufs=8) as xp,
            tc.tile_pool(name="x0p", bufs=4) as x0p,
            tc.tile_pool(name="singles", bufs=1) as singles,
            tc.tile_pool(name="psum", bufs=1, space="PSUM") as psum,
        ):
            aw_sb = singles.tile([128, 64 * KP], fp8)
            nc.sync.dma_start(out=aw_sb[:], in_=aw[:])
            aw_view = aw_sb.rearrange(
                "q (p w b m) -> q p w b m", p=KP, w=2, b=2
            )

            ps = {
                (s, j): psum.tile(
                    [16, 512], f32, tag=f"ps{s}{j}", name=f"ps{s}{j}"
                )
                for s in range(2)
                for j in range(JC)
            }

            def mm(out_ap, w, rhs, start, stop):
                nc.tensor.matmul(
                    out_ap,
                    w,
                    rhs,
                    start=start,
                    stop=stop,
                    perf_mode=mybir.MatmulPerfMode.DoubleRow,
                )

            def w_slices(p):
                return aw_view[:, p, 0], aw_view[:, p, 1]  # [128, 2, 16]

            # Segments: pair 0 is split into four 128 KB sub-tiles so the
            # first matmuls start as soon as the first sub-transfer lands
            # (warming the PE clock on real work); the last pair is split in
            # two so the tail square->matmul chain is short; the rest are
            # full 512 KB pair tiles.  (pair, j_lo, j_width)
            segments = [(0, j * 512, 512) for j in range(JC)]
            segments += [(p, 0, ROWS_PER_CORE) for p in range(1, KP - 1)]
            segments += [(KP - 1, h * 1024, 1024) for h in range(2)]

            for p, j_lo, j_w in segments:
                is_sub = j_w != ROWS_PER_CORE
                pool = x0p if is_sub else xp
                x_tile = pool.tile(
                    [128, 2, j_w], fp8, tag="x0" if is_sub else "x",
                    name=f"x_{p}_{j_lo}",
                )
                nc.sync.dma_start(
                    out=x_tile[:],
                    in_=xt_pairs[p][:, :, j_lo : j_lo + j_w],
                )
                sq_tile = pool.tile(
                    [128, 2, j_w], fp8, tag="sq0" if is_sub else "sq",
                    name=f"sq_{p}_{j_lo}",
                )
                # squares: scalar engine does chunk b=0, vector engine b=1.
                nc.scalar.activation(
                    out=sq_tile[:, 0, :],
                    in_=x_tile[:, 0, :],
                    func=mybir.ActivationFunctionType.Square,
                )
                nc.vector.tensor_mul(
                    sq_tile[:, 1, :], x_tile[:, 1, :], x_tile[:, 1, :]
                )
                s = p % 2
                first = p == s  # pair 0 starts set A, pair 1 starts set B
                last = p >= KP - 2  # pairs 6 (A) and 7 (B) close their sets
                w_x, w_q = w_slices(p)
                njc = j_w // 512
                for j in range(njc):
                    mm(
                        ps[s, j_lo // 512 + j][:],
                        w_x,
                        x_tile[:, :, j * 512 : (j + 1) * 512],
                        start=first,
                        stop=False,
                    )
                for j in range(njc):
                    mm(
                        ps[s, j_lo // 512 + j][:],
                        w_q,
                        sq_tile[:, :, j * 512 : (j + 1) * 512],
                        start=False,
                        stop=last,
                    )

            out_sb = singles.tile([2, 2 * ROWS_PER_CORE], f32)
            for s in range(2):
                for j in range(JC):
                    lo = s * ROWS_PER_CORE + j * 512
                    dst = out_sb[0:2, lo : lo + 512]
                    if (s + j) % 2 == 0:
                        nc.vector.tensor_copy(dst, ps[s, j][0:2, :])
                    else:
                        nc.scalar.copy(dst, ps[s, j][0:2, :])
            nc.sync.dma_start(out=out[:], in_=out_sb[:])

    nc.compile()
    return nc


def _get_nc():
    if "nc" not in _NC_CACHE:
        _NC_CACHE["nc"] = _build_bass()
    return _NC_CACHE["nc"]


def _make_in_maps(embed):
    x0 = embed[0].astype(np.float64)
    nrm0 = max(np.sqrt(np.dot(x0, x0)), NORM_EPS)
    a64 = x0 / nrm0 + PD_EPS
    a8 = a64.astype(FP8)

    # [128, p, wtype, b, m=16]: wtype 0 m=0 -> a_chunk, wtype 1 m=1 -> 1.0
    aw = np.zeros((128, KP, 2, 2, 16), FP8)
    for p in range(KP):
        for b in range(2):
            c = 2 * p + b
            aw[:, p, 0, b, 0] = a8[c * 128 : (c + 1) * 128]
            aw[:, p, 1, b, 1] = 1.0
    aw = aw.reshape(128, 64 * KP)

    in_maps = []
    for core in range(N_CORES):
        shard = embed[core * ROWS_PER_CORE : (core + 1) * ROWS_PER_CORE]
        xt = shard.T.astype(FP8)  # [DIM, ROWS_PER_CORE], C-contiguous
        in_maps.append({"xt": xt, "aw": aw})
    return in_maps, a64


def _epilogue(results, a64, labels):
    R = ROWS_PER_CORE
    adot = np.concatenate(
        [r["out"][0, :R].astype(np.float64) + r["out"][0, R:] for r in results]
    )
    ss = np.concatenate(
        [r["out"][1, :R].astype(np.float64) + r["out"][1, R:] for r in results]
    )

    nrm = np.maximum(np.sqrt(ss), NORM_EPS)
    t = adot / nrm  # a . e_j
    a2 = np.dot(a64, a64)
    d2 = np.maximum(a2 + 1.0 - 2.0 * t, 0.0)
    d = np.sqrt(d2)[1:]  # anchor row excluded, j = 1..n-1

    lab = labels.astype(np.float64)
    c = lab[1:] @ lab[0]
    ci = 1e-12 + c.sum()
    log_sim = -d / T
    ei = 1e-12 + np.exp(log_sim).sum()
    li = (-(c / ci) * (log_sim - np.log(ei))).sum()
    return np.asarray(li / N_ROWS, dtype=np.float32)


def _run(embed, labels, trace=False):
    embed = np.ascontiguousarray(np.asarray(embed, dtype=np.float32))
    labels = np.asarray(labels)
    assert embed.shape == (N_ROWS, DIM), embed.shape

    nc = _get_nc()
    in_maps, a64 = _make_in_maps(embed)
    kwargs = {"trace_cores": list(range(N_CORES))} if trace else {}
    res = run_bass_kernel_spmd(
        nc, in_maps, core_ids=list(range(N_CORES)), trace=trace, **kwargs
    )
    return _epilogue(res.results, a64, labels), res


def kernel(embed, labels):
    out, _ = _run(embed, labels, trace=False)
    return out


# revision 44
# speedup vs baseline: 1.1854x; 1.0475x over previous
"""Trainium2 Bass kernel for nn_CLloss (contrastive loss, anchor row 0).

Math (faithful to the torch/jax reference):
    e_j = x_j / max(||x_j||, 1e-12)          (row-normalize embed)
    d_j = ||(e_0 + 1e-6) - e_j||_2           (pairwise distance to anchor, j>=1)
    log_sim_j = -d_j / 0.1
    c_j = <labels_j, labels_0>
    Ci = 1e-12 + sum c_j ; Ei = 1e-12 + sum exp(log_sim_j)
    Li = sum -(c_j/Ci) * (log_sim_j - log Ei) ; loss = Li / n

With a = e_0 + 1e-6:  d_j^2 = ||a||^2 + 1 - 2*(a . x_j)/||x_j||, so the only
O(n*d) work is two per-row contractions over the feature dim: a.x_j and
sum_k x_jk^2.  Rows are sharded across 8 cores; each core gets its shard
TRANSPOSED (feature k on SBUF partitions, done on host) so the tensor engine
contracts over partitions:
  - a.x     via matmul(lhsT=[a | 0],  rhs=x)
  - sum x^2 via matmul(lhsT=[0 | 1],  rhs=square(x))
Both accumulate into the SAME psum tile (row 0 = a.x, row 1 = sum x^2)
across the feature chunks.  Squares are split between the scalar and vector
engines.  Inputs are cast to fp8 e4m3 on the host and matmuls use the
DoubleRow perf mode (256-deep contraction, 2 rows/cycle), which halves both
HBM traffic and tensor-engine time vs bf16.

Precision: the loss is a mean over 16k rows, so independent per-row rounding
noise averages down by ~sqrt(16384), and the fp8 quantization of the shared
anchor shifts all distances nearly uniformly — a shift that cancels exactly
between the sum(c*d)/T term and log(Ei).  Measured end-to-end error vs the
f32 reference is ~1e-5.  Device returns per-row (a.x, sum x^2); host does
the O(n) epilogue in f64.
"""

import ml_dtypes
import numpy as np

import concourse.bacc as bacc
import concourse.bass as bass
import concourse.tile as tile
from concourse import mybir
from concourse.bass_utils import run_bass_kernel_spmd
from concourse.tile import add_dep_helper

N_ROWS = 16384
DIM = 2048
N_CORES = 8
ROWS_PER_CORE = N_ROWS // N_CORES  # 2048
KC = DIM // 128  # 16 feature chunks of 128 partitions
KP = KC // 2  # 8 chunk-pairs (DoubleRow contracts 256 rows per matmul)
JC = ROWS_PER_CORE // 512  # 4 row chunks of 512 (psum bank = 512 f32)

PD_EPS = 1e-6
NORM_EPS = 1e-12
T = 0.1

FP8 = ml_dtypes.float8_e4m3

_NC_CACHE = {}


def _build_bass():
    # Bacc (not raw Bass): its compile() legalizes sync waits — walrus accepts
    # at most ONE wait per instruction, and Tile freely emits several.
    nc = bacc.Bacc()
    f32 = mybir.dt.float32
    fp8 = mybir.dt.float8e4
    xt = nc.dram_tensor("xt", [DIM, ROWS_PER_CORE], fp8, kind="ExternalInput")
    # Per chunk-pair p and pass wtype (0 = x, 1 = x^2), a [128, 2, 16] weight
    # block (DoubleRow ldweights requires the pair dim stride to be a
    # multiple of 16 elements).  Useful columns: m=0 carries a_chunk for the
    # x-pass, m=1 carries ones for the x^2-pass; the rest are zero.  Both
    # passes accumulate into the SAME psum tile: row 0 collects a.x only,
    # row 1 collects sum x^2 only.
    aw = nc.dram_tensor("aw", [128, 64 * KP], fp8, kind="ExternalInput")
    out = nc.dram_tensor("out", [2, ROWS_PER_CORE], f32, kind="ExternalOutput")

    # view as chunk-pairs: pair p, partition q, free [b, j] with b in {0,1}
    xt_pairs = xt.rearrange("(p b q) j -> p q b j", b=2, q=128)

    with tile.TileContext(nc) as tc:
        with (
            tc.tile_pool(name="xp", bufs=8) as xp,
            tc.tile_pool(name="x0p", bufs=4) as x0p,
            tc.tile_pool(name="singles", bufs=1) as singles,
            tc.tile_pool(name="psum", bufs=1, space="PSUM") as psum,
        ):
            aw_sb = singles.tile([128, 64 * KP], fp8)
            nc.sync.dma_start(out=aw_sb[:], in_=aw[:])
            aw_view = aw_sb.rearrange(
                "q (p w b m) -> q p w b m", p=KP, w=2, b=2
            )

            ps = [
                psum.tile([16, 512], f32, tag=f"ps{j}", name=f"ps{j}")
                for j in range(JC)
            ]

            # All matmuls are chained in program order on PE (order-only
            # deps, no semaphores) to keep execution deterministic.
            prev_mm = None

            def mm(out_ap, w, rhs, start, stop):
                nonlocal prev_mm
                inst = nc.tensor.matmul(
                    out_ap,
                    w,
                    rhs,
                    start=start,
                    stop=stop,
                    perf_mode=mybir.MatmulPerfMode.DoubleRow,
                ).ins
                if prev_mm is not None:
                    add_dep_helper(inst, prev_mm, reason="pe program order")
                prev_mm = inst

            def w_slices(p):
                return aw_view[:, p, 0], aw_view[:, p, 1]  # [128, 2, 16]

            # Segments: pair 0 is split into four 128 KB sub-tiles so the
            # first matmuls start as soon as the first sub-transfer lands
            # (warming the PE clock on real work); the last pair is split in
            # two so the tail square->matmul chain is short; the rest are
            # full 512 KB pair tiles.  (pair, j_lo, j_width)
            segments = [(0, j * 512, 512) for j in range(JC)]
            segments += [(p, 0, ROWS_PER_CORE) for p in range(1, KP - 1)]
            segments += [(KP - 1, h * 1024, 1024) for h in range(2)]

            for p, j_lo, j_w in segments:
                is_sub = j_w != ROWS_PER_CORE
                pool = x0p if is_sub else xp
                x_tile = pool.tile(
                    [128, 2, j_w], fp8, tag="x0" if is_sub else "x",
                    name=f"x_{p}_{j_lo}",
                )
                nc.sync.dma_start(
                    out=x_tile[:],
                    in_=xt_pairs[p][:, :, j_lo : j_lo + j_w],
                )
                sq_tile = pool.tile(
                    [128, 2, j_w], fp8, tag="sq0" if is_sub else "sq",
                    name=f"sq_{p}_{j_lo}",
                )
                # squares: scalar engine does chunk b=0, vector engine b=1.
                nc.scalar.activation(
                    out=sq_tile[:, 0, :],
                    in_=x_tile[:, 0, :],
                    func=mybir.ActivationFunctionType.Square,
                )
                nc.vector.tensor_mul(
                    sq_tile[:, 1, :], x_tile[:, 1, :], x_tile[:, 1, :]
                )
                w_x, w_q = w_slices(p)
                njc = j_w // 512
                for j in range(njc):
                    mm(
                        ps[j_lo // 512 + j][:],
                        w_x,
                        x_tile[:, :, j * 512 : (j + 1) * 512],
                        start=(p == 0),
                        stop=False,
                    )
                for j in range(njc):
                    mm(
                        ps[j_lo // 512 + j][:],
                        w_q,
                        sq_tile[:, :, j * 512 : (j + 1) * 512],
                        start=False,
                        stop=(p == KP - 1),
                    )

            out_sb = singles.tile([2, ROWS_PER_CORE], f32)
            for j in range(JC):
                dst = out_sb[0:2, j * 512 : (j + 1) * 512]
                if j % 2 == 0:
                    nc.vector.tensor_copy(dst, ps[j][0:2, :])
                else:
                    nc.scalar.copy(dst, ps[j][0:2, :])
            nc.sync.dma_start(out=out[:], in_=out_sb[:])

    nc.compile()
    return nc


def _get_nc():
    if "nc" not in _NC_CACHE:
        _NC_CACHE["nc"] = _build_bass()
    return _NC_CACHE["nc"]


def _make_in_maps(embed):
    x0 = embed[0].astype(np.float64)
    nrm0 = max(np.sqrt(np.dot(x0, x0)), NORM_EPS)
    a64 = x0 / nrm0 + PD_EPS
    a8 = a64.astype(FP8)

    # [128, p, wtype, b, m=16]: wtype 0 m=0 -> a_chunk, wtype 1 m=1 -> 1.0
    aw = np.zeros((128, KP, 2, 2, 16), FP8)
    for p in range(KP):
        for b in range(2):
            c = 2 * p + b
            aw[:, p, 0, b, 0] = a8[c * 128 : (c + 1) * 128]
            aw[:, p, 1, b, 1] = 1.0
    aw = aw.reshape(128, 64 * KP)

    in_maps = []
    for core in range(N_CORES):
        shard = embed[core * ROWS_PER_CORE : (core + 1) * ROWS_PER_CORE]
        xt = shard.T.astype(FP8)  # [DIM, ROWS_PER_CORE], C-contiguous
        in_maps.append({"xt": xt, "aw": aw})
    return in_maps, a64


def _epilogue(results, a64, labels):
    adot = np.concatenate([r["out"][0] for r in results]).astype(np.float64)
    ss = np.concatenate([r["out"][1] for r in results]).astype(np.float64)

    nrm = np.maximum(np.sqrt(ss), NORM_EPS)
    t = adot / nrm  # a . e_j
    a2 = np.dot(a64, a64)
    d2 = np.maximum(a2 + 1.0 - 2.0 * t, 0.0)
    d = np.sqrt(d2)[1:]  # anchor row excluded, j = 1..n-1

    lab = labels.astype(np.float64)
    c = lab[1:] @ lab[0]
    ci = 1e-12 + c.sum()
    log_sim = -d / T
    ei = 1e-12 + np.exp(log_sim).sum()
    li = (-(c / ci) * (log_sim - np.log(ei))).sum()
    return np.asarray(li / N_ROWS, dtype=np.float32)


def _run(embed, labels, trace=False):
    embed = np.ascontiguousarray(np.asarray(embed, dtype=np.float32))
    labels = np.asarray(labels)
    assert embed.shape == (N_ROWS, DIM), embed.shape

    nc = _get_nc()
    in_maps, a64 = _make_in_maps(embed)
    kwargs = {"trace_cores": list(range(N_CORES))} if trace else {}
    res = run_bass_kernel_spmd(
        nc, in_maps, core_ids=list(range(N_CORES)), trace=trace, **kwargs
    )
    return _epilogue(res.results, a64, labels), res


def kernel(embed, labels):
    out, _ = _run(embed, labels, trace=False)
    return out


# revision 45
# speedup vs baseline: 1.1868x; 1.0012x over previous
"""Trainium2 Bass kernel for nn_CLloss (contrastive loss, anchor row 0).

Math (faithful to the torch/jax reference):
    e_j = x_j / max(||x_j||, 1e-12)          (row-normalize embed)
    d_j = ||(e_0 + 1e-6) - e_j||_2           (pairwise distance to anchor, j>=1)
    log_sim_j = -d_j / 0.1
    c_j = <labels_j, labels_0>
    Ci = 1e-12 + sum c_j ; Ei = 1e-12 + sum exp(log_sim_j)
    Li = sum -(c_j/Ci) * (log_sim_j - log Ei) ; loss = Li / n

With a = e_0 + 1e-6:  d_j^2 = ||a||^2 + 1 - 2*(a . x_j)/||x_j||, so the only
O(n*d) work is two per-row contractions over the feature dim: a.x_j and
sum_k x_jk^2.  Rows are sharded across 8 cores; each core gets its shard
TRANSPOSED (feature k on SBUF partitions, done on host) so the tensor engine
contracts over partitions:
  - a.x     via matmul(lhsT=[a | 0],  rhs=x)
  - sum x^2 via matmul(lhsT=[0 | 1],  rhs=square(x))
Both accumulate into the SAME psum tile (row 0 = a.x, row 1 = sum x^2)
across the feature chunks.  Squares are split between the scalar and vector
engines.  Inputs are cast to fp8 e4m3 on the host and matmuls use the
DoubleRow perf mode (256-deep contraction, 2 rows/cycle), which halves both
HBM traffic and tensor-engine time vs bf16.

Precision: the loss is a mean over 16k rows, so independent per-row rounding
noise averages down by ~sqrt(16384), and the fp8 quantization of the shared
anchor shifts all distances nearly uniformly — a shift that cancels exactly
between the sum(c*d)/T term and log(Ei).  Measured end-to-end error vs the
f32 reference is ~1e-5.  Device returns per-row (a.x, sum x^2); host does
the O(n) epilogue in f64.
"""

import ml_dtypes
import numpy as np

import concourse.bacc as bacc
import concourse.tile as tile
from concourse import mybir
from concourse.bass_utils import run_bass_kernel_spmd
from concourse.tile import add_dep_helper

N_ROWS = 16384
DIM = 2048
N_CORES = 8
ROWS_PER_CORE = N_ROWS // N_CORES  # 2048
KC = DIM // 128  # 16 feature chunks of 128 partitions
KP = KC // 2  # 8 chunk-pairs (DoubleRow contracts 256 rows per matmul)
JC = ROWS_PER_CORE // 512  # 4 row chunks of 512 (psum bank = 512 f32)

PD_EPS = 1e-6
NORM_EPS = 1e-12
T = 0.1

FP8 = ml_dtypes.float8_e4m3

_NC_CACHE = {}


def _build_bass():
    # Bacc (not raw Bass): its compile() legalizes sync waits — walrus accepts
    # at most ONE wait per instruction, and Tile freely emits several.
    nc = bacc.Bacc()
    f32 = mybir.dt.float32
    fp8 = mybir.dt.float8e4
    xt = nc.dram_tensor("xt", [DIM, ROWS_PER_CORE], fp8, kind="ExternalInput")
    # Per chunk-pair p and pass wtype (0 = x, 1 = x^2), a [128, 2, 16] weight
    # block (DoubleRow ldweights requires the pair dim stride to be a
    # multiple of 16 elements).  Useful columns: m=0 carries a_chunk for the
    # x-pass, m=1 carries ones for the x^2-pass; the rest are zero.  Both
    # passes accumulate into the SAME psum tile: row 0 collects a.x only,
    # row 1 collects sum x^2 only.
    aw = nc.dram_tensor("aw", [128, 64 * KP], fp8, kind="ExternalInput")
    out = nc.dram_tensor("out", [2, ROWS_PER_CORE], f32, kind="ExternalOutput")

    # view as chunk-pairs: pair p, partition q, free [b, j] with b in {0,1}
    xt_pairs = xt.rearrange("(p b q) j -> p q b j", b=2, q=128)

    with tile.TileContext(nc) as tc:
        with (
            tc.tile_pool(name="xp", bufs=8) as xp,
            tc.tile_pool(name="x0p", bufs=4) as x0p,
            tc.tile_pool(name="singles", bufs=1) as singles,
            tc.tile_pool(name="psum", bufs=1, space="PSUM") as psum,
        ):
            aw_sb = singles.tile([128, 64 * KP], fp8)
            nc.sync.dma_start(out=aw_sb[:], in_=aw[:])
            aw_view = aw_sb.rearrange(
                "q (p w b m) -> q p w b m", p=KP, w=2, b=2
            )

            ps = [
                psum.tile([16, 512], f32, tag=f"ps{j}", name=f"ps{j}")
                for j in range(JC)
            ]

            # All matmuls are chained in program order on PE (order-only
            # deps, no semaphores) to keep execution deterministic.
            prev_mm = None

            def mm(out_ap, w, rhs, start, stop):
                nonlocal prev_mm
                inst = nc.tensor.matmul(
                    out_ap,
                    w,
                    rhs,
                    start=start,
                    stop=stop,
                    perf_mode=mybir.MatmulPerfMode.DoubleRow,
                ).ins
                if prev_mm is not None:
                    add_dep_helper(inst, prev_mm, reason="pe program order")
                prev_mm = inst

            def w_slices(p):
                return aw_view[:, p, 0], aw_view[:, p, 1]  # [128, 2, 16]

            # Segments: pair 0 is split into four 128 KB sub-tiles so the
            # first matmuls start as soon as the first sub-transfer lands
            # (warming the PE clock on real work); the last pair is split in
            # two so the tail square->matmul chain is short; the rest are
            # full 512 KB pair tiles.  (pair, j_lo, j_width)
            segments = [(0, j * 512, 512) for j in range(JC)]
            segments += [(p, 0, ROWS_PER_CORE) for p in range(1, KP - 1)]
            segments += [(KP - 1, h * 1024, 1024) for h in range(2)]

            for p, j_lo, j_w in segments:
                is_sub = j_w != ROWS_PER_CORE
                pool = x0p if is_sub else xp
                x_tile = pool.tile(
                    [128, 2, j_w], fp8, tag="x0" if is_sub else "x",
                    name=f"x_{p}_{j_lo}",
                )
                nc.sync.dma_start(
                    out=x_tile[:],
                    in_=xt_pairs[p][:, :, j_lo : j_lo + j_w],
                )
                sq_tile = pool.tile(
                    [128, 2, j_w], fp8, tag="sq0" if is_sub else "sq",
                    name=f"sq_{p}_{j_lo}",
                )
                # squares: scalar engine does chunk b=0, vector engine b=1.
                nc.scalar.activation(
                    out=sq_tile[:, 0, :],
                    in_=x_tile[:, 0, :],
                    func=mybir.ActivationFunctionType.Square,
                )
                nc.vector.tensor_mul(
                    sq_tile[:, 1, :], x_tile[:, 1, :], x_tile[:, 1, :]
                )
                w_x, w_q = w_slices(p)
                njc = j_w // 512
                for j in range(njc):
                    mm(
                        ps[j_lo // 512 + j][:],
                        w_x,
                        x_tile[:, :, j * 512 : (j + 1) * 512],
                        start=(p == 0),
                        stop=False,
                    )
                for j in range(njc):
                    mm(
                        ps[j_lo // 512 + j][:],
                        w_q,
                        sq_tile[:, :, j * 512 : (j + 1) * 512],
                        start=False,
                        stop=(p == KP - 1),
                    )

            out_sb = singles.tile([2, ROWS_PER_CORE], f32)
            for j in range(JC):
                dst = out_sb[0:2, j * 512 : (j + 1) * 512]
                if j % 2 == 0:
                    nc.vector.tensor_copy(dst, ps[j][0:2, :])
                else:
                    nc.scalar.copy(dst, ps[j][0:2, :])
            nc.sync.dma_start(out=out[:], in_=out_sb[:])

    nc.compile()
    return nc


def _get_nc():
    if "nc" not in _NC_CACHE:
        _NC_CACHE["nc"] = _build_bass()
    return _NC_CACHE["nc"]


def _make_in_maps(embed):
    x0 = embed[0].astype(np.float64)
    nrm0 = max(np.sqrt(np.dot(x0, x0)), NORM_EPS)
    a64 = x0 / nrm0 + PD_EPS
    a8 = a64.astype(FP8)

    # [128, p, wtype, b, m=16]: wtype 0 m=0 -> a_chunk, wtype 1 m=1 -> 1.0
    aw = np.zeros((128, KP, 2, 2, 16), FP8)
    for p in range(KP):
        for b in range(2):
            c = 2 * p + b
            aw[:, p, 0, b, 0] = a8[c * 128 : (c + 1) * 128]
            aw[:, p, 1, b, 1] = 1.0
    aw = aw.reshape(128, 64 * KP)

    in_maps = []
    for core in range(N_CORES):
        shard = embed[core * ROWS_PER_CORE : (core + 1) * ROWS_PER_CORE]
        xt = shard.T.astype(FP8)  # [DIM, ROWS_PER_CORE], C-contiguous
        in_maps.append({"xt": xt, "aw": aw})
    return in_maps, a64


def _epilogue(results, a64, labels):
    adot = np.concatenate([r["out"][0] for r in results]).astype(np.float64)
    ss = np.concatenate([r["out"][1] for r in results]).astype(np.float64)

    nrm = np.maximum(np.sqrt(ss), NORM_EPS)
    t = adot / nrm  # a . e_j
    a2 = np.dot(a64, a64)
    d2 = np.maximum(a2 + 1.0 - 2.0 * t, 0.0)
    d = np.sqrt(d2)[1:]  # anchor row excluded, j = 1..n-1

    lab = labels.astype(np.float64)
    c = lab[1:] @ lab[0]
    ci = 1e-12 + c.sum()
    log_sim = -d / T
    ei = 1e-12 + np.exp(log_sim).sum()
    li = (-(c / ci) * (log_sim - np.log(ei))).sum()
    return np.asarray(li / N_ROWS, dtype=np.float32)


def _run(embed, labels, trace=False):
    embed = np.ascontiguousarray(np.asarray(embed, dtype=np.float32))
    labels = np.asarray(labels)
    assert embed.shape == (N_ROWS, DIM), embed.shape

    nc = _get_nc()
    in_maps, a64 = _make_in_maps(embed)
    kwargs = {"trace_cores": list(range(N_CORES))} if trace else {}
    res = run_bass_kernel_spmd(
        nc, in_maps, core_ids=list(range(N_CORES)), trace=trace, **kwargs
    )
    return _epilogue(res.results, a64, labels), res


def kernel(embed, labels):
    out, _ = _run(embed, labels, trace=False)
    return out
